# revision 37
# baseline (speedup 1.0000x reference)
"""Causal multi-head attention on 8 TRN2 NeuronCores.

Sharding: 8 cores = 4 batches x 2 head-groups (8 heads each).
Each core computes q/k/v projections for its head group, flash-style
causal attention in S^T layout ([k, q], softmax across partitions via a
ones-column in the PV matmul), and a partial output projection
(row-split Wo).  Host sums the two partial outputs per batch.

All matmuls run in bf16 with fp32 PSUM accumulation.  Activations are
fed to the device pre-transposed ([E, L]) and pre-tiled so every DMA
moves >=4KB contiguous per partition.
"""

import sys

sys.path.insert(0, "/opt/trn_rl_repo")

from contextlib import ExitStack

import numpy as np
import ml_dtypes

import concourse.bass as bass
import concourse.mybir as mybir
import concourse.tile as tile
from concourse import bacc
from concourse.bass_utils import run_bass_kernel_spmd

BF16 = mybir.dt.bfloat16
F32 = mybir.dt.float32
F8 = mybir.dt.float8e4

B, L, E, H, D = 4, 2048, 1024, 16, 64
NCORES = 8
HPC = H // 2          # heads per core (8)
DH = HPC * D          # per-core projected dim (512)
LB = 512              # q-block width
NLB = L // LB         # 4
ET = E // 128         # 8 contraction tiles for projections
EG = 2                # e-tile groups per DMA (ET // 4)
MT = DH // 128        # 4 dout tiles
KT = L // 128         # 16 key tiles
EXP_SCALE = 1.0 / np.sqrt(D)


def _build(uniform_pad=True):
    nc = bacc.Bacc("TRN2", target_bir_lowering=False, debug=False,
                   num_devices=NCORES)

    qT_in = nc.dram_tensor("qT", [EG, NLB, 128, ET // EG, LB], BF16, kind="ExternalInput").ap()
    kT_in = nc.dram_tensor("kT", [EG, NLB, 128, ET // EG, LB], BF16, kind="ExternalInput").ap()
    vT_in = nc.dram_tensor("vT", [EG, NLB, 128, ET // EG, LB], BF16, kind="ExternalInput").ap()
    wq_in = nc.dram_tensor("wq", [128, MT, ET, 128], BF16, kind="ExternalInput").ap()
    wk_in = nc.dram_tensor("wk", [128, ET, DH], BF16, kind="ExternalInput").ap()
    wv_in = nc.dram_tensor("wv", [128, ET, DH], BF16, kind="ExternalInput").ap()
    wo_in = nc.dram_tensor("wo", [128, MT, E], BF16, kind="ExternalInput").ap()
    bias_in = nc.dram_tensor("bias", [128, KT], F32, kind="ExternalInput").ap()
    out_ext = nc.dram_tensor("out", [KT, 128, 2, LB], BF16, kind="ExternalOutput").ap()

    with tile.TileContext(nc) as tc, ExitStack() as ctx:
        wpool = ctx.enter_context(tc.tile_pool(name="weights", bufs=1))
        ppool = ctx.enter_context(tc.tile_pool(name="persist", bufs=1))
        xpool = ctx.enter_context(tc.tile_pool(name="xT", bufs=3))
        pTpool = ctx.enter_context(tc.tile_pool(name="pT", bufs=3))
        opool = ctx.enter_context(tc.tile_pool(name="outsb", bufs=3))
        rpool = ctx.enter_context(tc.tile_pool(name="rnorm", bufs=6))
        ps_proj = ctx.enter_context(tc.tile_pool(name="ps_proj", bufs=2, space="PSUM"))
        ps_sp = ctx.enter_context(tc.tile_pool(name="ps_sp", bufs=2, space="PSUM"))
        ps_oacc = ctx.enter_context(tc.tile_pool(name="ps_oacc", bufs=2, space="PSUM"))

        # ---- resident weights (one DMA each, >=4KB/partition) ------------
        wq_sb = wpool.tile([128, MT, ET, 128], BF16, tag="wq")
        wk_sb = wpool.tile([128, ET, DH], BF16, tag="wk")
        wv_sb = wpool.tile([128, ET, DH], BF16, tag="wv")
        wo_sb = wpool.tile([128, MT, E], BF16, tag="wo")
        bias_sb = wpool.tile([128, KT], F32, tag="bias")

        # ---- resident activations ---------------------------------------
        qT_sb = ppool.tile([128, MT, L], BF16, tag="qT")
        kT_sb = ppool.tile([128, MT, L], BF16, tag="kT")
        v_sb = ppool.tile([128, KT, HPC, D + 1], BF16, tag="v")
        oT_sb = ppool.tile([128, MT, L], BF16, tag="oT")
        nc.gpsimd.memset(v_sb[:, :, :, D:D + 1], 1.0)
        # lower-triangular keep-mask (keep q' >= k'), bf16 ones/zeros,
        # duplicated along a middle dim so one mul covers both heads
        trimask = wpool.tile([128, 2, 128], BF16, tag="trimask")
        nc.gpsimd.memset(trimask[:], 1.0)
        nc.gpsimd.affine_select(
            out=trimask[:], in_=trimask[:],
            compare_op=mybir.AluOpType.is_ge,
            fill=0.0, base=0,
            pattern=[[0, 2], [1, 128]],
            channel_multiplier=-1)

        # ---- pipeline units ---------------------------------------------
        xcache = {}

        def get_xtiles(lb):
            if lb not in xcache:
                xq, xk, xv = [], [], []
                for eg in range(EG):
                    tq = xpool.tile([128, ET // EG, LB], BF16, tag="xq", name="xq")
                    nc.sync.dma_start(tq[:], qT_in[eg, lb])
                    xq.append(tq)
                    if lb == 0:
                        nc.scalar.dma_start(wq_sb[:, 2 * eg:2 * eg + 2], wq_in[:, 2 * eg:2 * eg + 2])
                for eg in range(EG):
                    tk = xpool.tile([128, ET // EG, LB], BF16, tag="xk", name="xk")
                    nc.gpsimd.dma_start(tk[:], kT_in[eg, lb])
                    xk.append(tk)
                    tv = xpool.tile([128, ET // EG, LB], BF16, tag="xv", name="xv")
                    nc.gpsimd.dma_start(tv[:], vT_in[eg, lb])
                    xv.append(tv)
                if lb == 0:
                    nc.scalar.dma_start(wk_sb[:], wk_in[:])
                    nc.scalar.dma_start(wv_sb[:], wv_in[:])
                    nc.scalar.dma_start(bias_sb[:], bias_in[:])
                    nc.scalar.dma_start(wo_sb[:], wo_in[:])
                xcache[lb] = (xq, xk, xv)
            return xcache[lb]

        def proj_qk_unit(lb, which, m):
            def fn():
                xq, xk, xv = get_xtiles(lb)
                dst = (qT_sb, kT_sb)[which]
                xs = (xq, xk)[which]
                w_sb = (wq_sb, wk_sb)[which]
                ps = ps_proj.tile([128, LB], F32, tag="ps_proj", name="psp")
                for t in range(ET):
                    lhsT = (w_sb[:, m, t, :] if which == 0
                            else w_sb[:, t, m * 128:(m + 1) * 128])
                    nc.tensor.matmul(
                        ps[:],
                        lhsT=lhsT,
                        rhs=xs[t // 4][:, t % 4, :],
                        start=(t == 0), stop=(t == ET - 1))
                nc.vector.tensor_copy(dst[:, m, lb * LB:(lb + 1) * LB], ps[:])
            return fn

        def proj_v_unit(lb, lt):
            def fn():
                xq, xk, xv = get_xtiles(lb)
                ps = ps_proj.tile([128, HPC, D], F32, tag="ps_proj", name="psv")
                for t in range(ET):
                    nc.tensor.matmul(
                        ps[:],
                        lhsT=xv[t // 4][:, t % 4, lt * 128:(lt + 1) * 128],
                        rhs=wv_sb[:, t, :],
                        start=(t == 0), stop=(t == ET - 1))
                nc.vector.tensor_copy(v_sb[:, lb * 4 + lt, :, 0:D], ps[:])
            return fn

        def oproj_unit(lb, lt):
            def fn():
                l_tile = lb * 4 + lt
                ob = opool.tile([128, 2, LB], BF16, tag="ob", name="ob")
                for e in range(2):
                    ps = ps_proj.tile([128, LB], F32, tag="ps_proj", name="pso")
                    for r in range(MT):
                        nc.tensor.matmul(
                            ps[:],
                            lhsT=oT_sb[:, r, l_tile * 128:(l_tile + 1) * 128],
                            rhs=wo_sb[:, r, e * LB:(e + 1) * LB],
                            start=(r == 0), stop=(r == MT - 1))
                    nc.vector.tensor_copy(ob[:, e, :], ps[:])
                nc.sync.dma_start(out_ext[l_tile], ob[:])
            return fn

        def proj_block(lb):
            for which in range(2):
                for m in range(MT):
                    proj_qk_unit(lb, which, m)()
            for lt in range(LB // 128):
                proj_v_unit(lb, lt)()

        def attn_pair(hp, j, filler):
            nki = 4 * j + 4
            oaccs = [ps_oacc.tile([D + 1, LB], F32, tag="oacc", name="oacc")
                     for _ in range(2)]

            def emit_pv(ki, pT):
                sk = ki - 4 * j
                x0 = 128 * sk if sk >= 0 else 0
                for hi in range(2):
                    nc.tensor.matmul(
                        oaccs[hi][:, x0:LB],
                        lhsT=v_sb[:, ki, 2 * hp + hi, :],
                        rhs=pT[:, hi, x0:LB],
                        start=(ki == 0), stop=(ki == nki - 1))

            prev = None
            for ki in range(nki):
                sk = ki - 4 * j
                x0 = 128 * sk if sk >= 0 else 0
                sp = ps_sp.tile([128, 2, LB], F32, tag="sp", name="sp")
                for hi in range(2):
                    p0 = hi * 64
                    nc.tensor.matmul(
                        sp[:, hi, x0:LB],
                        lhsT=kT_sb[p0:p0 + 64, hp, ki * 128:(ki + 1) * 128],
                        rhs=qT_sb[p0:p0 + 64, hp, j * LB + x0:(j + 1) * LB],
                        start=True, stop=True, tile_position=(p0, 0))
                pT = pTpool.tile([128, 2, LB], BF16, tag="pT", name="pT")
                if not uniform_pad:
                    nc.vector.tensor_scalar_add(
                        sp[:, :, x0:LB], sp[:, :, x0:LB],
                        bias_sb[:, ki:ki + 1])
                nc.scalar.activation(pT[:, :, x0:LB], sp[:, :, x0:LB],
                                     mybir.ActivationFunctionType.Exp,
                                     bias=0.0,
                                     scale=float(EXP_SCALE))
                if sk >= 0:
                    nc.vector.tensor_mul(
                        pT[:, :, x0:x0 + 128],
                        pT[:, :, x0:x0 + 128], trimask[:])
                if prev is not None:
                    emit_pv(*prev)
                prev = (ki, pT)
                filler()
            emit_pv(*prev)
            osbs = []
            for hi in range(2):
                osb = rpool.tile([D + 1, LB], F32, tag="osb", name="osb")
                nc.vector.tensor_copy(osb[:], oaccs[hi][:])
                osbs.append(osb)

            def norm_fn():
                for hi in range(2):
                    osb = osbs[hi]
                    rsum = rpool.tile([1, LB], F32, tag="rsum", name="rsum")
                    nc.vector.tensor_copy(rsum[:], osb[D:D + 1, :])
                    rinv1 = rpool.tile([1, LB], F32, tag="rinv1", name="rinv1")
                    nc.vector.reciprocal_approx_fast(rinv1[:], rsum[:])
                    rinv = rpool.tile([64, LB], F32, tag="rinv", name="rinv")
                    nc.gpsimd.partition_broadcast(rinv[:], rinv1[:])
                    p0 = hi * 64
                    nc.vector.tensor_mul(
                        oT_sb[p0:p0 + 64, hp, j * LB:(j + 1) * LB],
                        osb[0:D, :], rinv[:])
            return norm_fn

        proj_block(0)
        pending_norms = []
        for j in range(NLB):
            units = []
            barriers = {}
            if j < 2:
                for m in range(MT):
                    units.append(proj_qk_unit(j + 1, 0, m))
                    units.append(proj_qk_unit(j + 1, 1, m))
                for lt in range(LB // 128):
                    units.append(proj_v_unit(j + 1, lt))
            elif j == 2:
                for lt in range(LB // 128):
                    units.append(proj_v_unit(3, lt))
                units.append(proj_qk_unit(3, 0, 0))
                units.append(proj_qk_unit(3, 1, 0))
                for lt in range(LB // 128):
                    units.append(oproj_unit(0, lt))
            else:
                for m in range(1, MT):
                    units.append(proj_qk_unit(3, 0, m))
                    units.append(proj_qk_unit(3, 1, m))
                for lt in range(LB // 128):
                    units.append(oproj_unit(1, lt))
                for lt in range(LB // 128):
                    units.append(oproj_unit(2, lt))
                barriers = {1: 2, 2: 4, 3: 6}

            state = {"slot": 0, "done": 0}
            total_slots = (HPC // 2) * (4 * j + 4)

            def filler():
                state["slot"] += 1
                want = len(units) * state["slot"] // total_slots
                while state["done"] < want:
                    units[state["done"]]()
                    state["done"] += 1

            for hp in range(HPC // 2):
                while state["done"] < barriers.get(hp, 0):
                    units[state["done"]]()
                    state["done"] += 1
                nf = attn_pair(hp, j, filler)
                pending_norms.append(nf)
                if len(pending_norms) > 2:
                    pending_norms.pop(0)()
            while state["done"] < len(units):
                units[state["done"]]()
                state["done"] += 1
            while pending_norms:
                pending_norms.pop(0)()
        for lt in range(LB // 128):
            oproj_unit(NLB - 1, lt)()

    nc.compile()
    return nc


_CACHE = {}


def _get_nc(uniform_pad=True):
    key = ("nc", uniform_pad)
    if key not in _CACHE:
        _CACHE[key] = _build(uniform_pad)
    return _CACHE[key]


def _prepare_in_maps(query, key, value, pad_mask, Wq, Wk, Wv, Wo):
    bf = ml_dtypes.bfloat16
    f8 = ml_dtypes.float8_e4m3fn
    query = np.asarray(query, np.float32)
    key = np.asarray(key, np.float32)
    value = np.asarray(value, np.float32)
    pad_mask = np.asarray(pad_mask)
    Wq = np.asarray(Wq, np.float32)
    Wk = np.asarray(Wk, np.float32)
    Wv = np.asarray(Wv, np.float32)
    Wo = np.asarray(Wo, np.float32)

    def tile_act(x):
        # [L, E] -> [E, L] -> [EG, NLB, 128, ET//EG, LB], 4KB/partition chunks
        xt = x.T.reshape(EG, ET // EG, 128, NLB, LB).transpose(0, 3, 2, 1, 4)
        return np.ascontiguousarray(xt.astype(bf))

    per_batch = []
    for b in range(B):
        bias = np.where(pad_mask[b] != 0, 0.0, -30000.0 / EXP_SCALE).astype(np.float32)
        bias = np.ascontiguousarray(bias.reshape(KT, 128).T)
        per_batch.append({
            "qT": tile_act(query[b]),
            "kT": tile_act(key[b]),
            "vT": tile_act(value[b]),
            "bias": bias,
        })

    per_group = []
    for g in range(2):
        sl = slice(g * DH, (g + 1) * DH)
        per_group.append({
            "wq": np.ascontiguousarray(
                Wq[:, sl].astype(bf).reshape(ET, 128, MT, 128).transpose(1, 2, 0, 3)),
            "wk": np.ascontiguousarray(
                Wk[:, sl].astype(bf).reshape(ET, 128, DH).transpose(1, 0, 2)),
            "wv": np.ascontiguousarray(
                Wv[:, sl].astype(bf).reshape(ET, 128, DH).transpose(1, 0, 2)),
            "wo": np.ascontiguousarray(
                Wo[sl, :].astype(bf).reshape(MT, 128, E).transpose(1, 0, 2)),
        })

    in_maps = []
    for b in range(B):
        for g in range(2):
            m = dict(per_batch[b])
            m.update(per_group[g])
            in_maps.append(m)
    return in_maps


def _combine(results):
    out = np.empty((B, L, E), np.float32)
    for b in range(B):
        acc = (results[2 * b]["out"].astype(np.float32)
               + results[2 * b + 1]["out"].astype(np.float32))
        out[b] = acc.reshape(L, E)
    return out


def kernel(query, key, value, pad_mask, Wq, Wk, Wv, Wo):
    nc = _get_nc(bool(np.all(np.asarray(pad_mask) != 0)))
    in_maps = _prepare_in_maps(query, key, value, pad_mask, Wq, Wk, Wv, Wo)
    res = run_bass_kernel_spmd(nc, in_maps, core_ids=list(range(NCORES)))
    return _combine(res.results)


# revision 38
# speedup vs baseline: 1.0057x; 1.0057x over previous
"""Causal multi-head attention on 8 TRN2 NeuronCores.

Sharding: 8 cores = 4 batches x 2 head-groups (8 heads each).
Each core computes q/k/v projections for its head group, flash-style
causal attention in S^T layout ([k, q], softmax across partitions via a
ones-column in the PV matmul), and a partial output projection
(row-split Wo).  Host sums the two partial outputs per batch.

All matmuls run in bf16 with fp32 PSUM accumulation.  Activations are
fed to the device pre-transposed ([E, L]) and pre-tiled so every DMA
moves >=4KB contiguous per partition.
"""

import sys

sys.path.insert(0, "/opt/trn_rl_repo")

from contextlib import ExitStack

import numpy as np
import ml_dtypes

import concourse.bass as bass
import concourse.mybir as mybir
import concourse.tile as tile
from concourse import bacc
from concourse.bass_utils import run_bass_kernel_spmd

BF16 = mybir.dt.bfloat16
F32 = mybir.dt.float32
F8 = mybir.dt.float8e4

B, L, E, H, D = 4, 2048, 1024, 16, 64
NCORES = 8
HPC = H // 2          # heads per core (8)
DH = HPC * D          # per-core projected dim (512)
LB = 512              # q-block width
NLB = L // LB         # 4
ET = E // 128         # 8 contraction tiles for projections
EG = 2                # e-tile groups per DMA (ET // 4)
MT = DH // 128        # 4 dout tiles
KT = L // 128         # 16 key tiles
EXP_SCALE = 1.0 / np.sqrt(D)


def _build(uniform_pad=True):
    nc = bacc.Bacc("TRN2", target_bir_lowering=False, debug=False,
                   num_devices=NCORES)

    qT_in = nc.dram_tensor("qT", [EG, NLB, 128, ET // EG, LB], BF16, kind="ExternalInput").ap()
    kT_in = nc.dram_tensor("kT", [EG, NLB, 128, ET // EG, LB], BF16, kind="ExternalInput").ap()
    vT_in = nc.dram_tensor("vT", [EG, NLB, 128, ET // EG, LB], BF16, kind="ExternalInput").ap()
    wq_in = nc.dram_tensor("wq", [128, MT, ET, 128], BF16, kind="ExternalInput").ap()
    wk_in = nc.dram_tensor("wk", [128, ET, DH], BF16, kind="ExternalInput").ap()
    wv_in = nc.dram_tensor("wv", [128, ET, DH], BF16, kind="ExternalInput").ap()
    wo_in = nc.dram_tensor("wo", [128, MT, E], BF16, kind="ExternalInput").ap()
    bias_in = nc.dram_tensor("bias", [128, KT], F32, kind="ExternalInput").ap()
    out_ext = nc.dram_tensor("out", [KT, 128, 2, LB], BF16, kind="ExternalOutput").ap()

    with tile.TileContext(nc) as tc, ExitStack() as ctx:
        wpool = ctx.enter_context(tc.tile_pool(name="weights", bufs=1))
        ppool = ctx.enter_context(tc.tile_pool(name="persist", bufs=1))
        xpool = ctx.enter_context(tc.tile_pool(name="xT", bufs=3))
        pTpool = ctx.enter_context(tc.tile_pool(name="pT", bufs=3))
        opool = ctx.enter_context(tc.tile_pool(name="outsb", bufs=3))
        rpool = ctx.enter_context(tc.tile_pool(name="rnorm", bufs=6))
        ps_proj = ctx.enter_context(tc.tile_pool(name="ps_proj", bufs=2, space="PSUM"))
        ps_sp = ctx.enter_context(tc.tile_pool(name="ps_sp", bufs=2, space="PSUM"))
        ps_oacc = ctx.enter_context(tc.tile_pool(name="ps_oacc", bufs=2, space="PSUM"))

        # ---- resident weights (one DMA each, >=4KB/partition) ------------
        wq_sb = wpool.tile([128, MT, ET, 128], BF16, tag="wq")
        wk_sb = wpool.tile([128, ET, DH], BF16, tag="wk")
        wv_sb = wpool.tile([128, ET, DH], BF16, tag="wv")
        wo_sb = wpool.tile([128, MT, E], BF16, tag="wo")
        bias_sb = wpool.tile([128, KT], F32, tag="bias")

        # ---- resident activations ---------------------------------------
        qT_sb = ppool.tile([128, MT, L], BF16, tag="qT")
        kT_sb = ppool.tile([128, MT, L], BF16, tag="kT")
        v_sb = ppool.tile([128, KT, HPC, D + 1], BF16, tag="v")
        oT_sb = ppool.tile([128, MT, L], BF16, tag="oT")
        nc.gpsimd.memset(v_sb[:, :, :, D:D + 1], 1.0)
        # lower-triangular keep-mask (keep q' >= k'), bf16 ones/zeros,
        # duplicated along a middle dim so one mul covers both heads
        trimask = wpool.tile([128, 2, 128], BF16, tag="trimask")
        nc.gpsimd.memset(trimask[:], 1.0)
        nc.gpsimd.affine_select(
            out=trimask[:], in_=trimask[:],
            compare_op=mybir.AluOpType.is_ge,
            fill=0.0, base=0,
            pattern=[[0, 2], [1, 128]],
            channel_multiplier=-1)

        # ---- pipeline units ---------------------------------------------
        xcache = {}

        def get_xtiles(lb):
            if lb not in xcache:
                xq, xk, xv = [], [], []
                for eg in range(EG):
                    tq = xpool.tile([128, ET // EG, LB], BF16, tag="xq", name="xq")
                    nc.sync.dma_start(tq[:], qT_in[eg, lb])
                    xq.append(tq)
                    if lb == 0:
                        nc.scalar.dma_start(wq_sb[:, 2 * eg:2 * eg + 2], wq_in[:, 2 * eg:2 * eg + 2])
                for eg in range(EG):
                    tk = xpool.tile([128, ET // EG, LB], BF16, tag="xk", name="xk")
                    nc.gpsimd.dma_start(tk[:], kT_in[eg, lb])
                    xk.append(tk)
                    tv = xpool.tile([128, ET // EG, LB], BF16, tag="xv", name="xv")
                    nc.gpsimd.dma_start(tv[:], vT_in[eg, lb])
                    xv.append(tv)
                if lb == 0:
                    nc.scalar.dma_start(wk_sb[:], wk_in[:])
                    nc.scalar.dma_start(wv_sb[:], wv_in[:])
                    nc.scalar.dma_start(bias_sb[:], bias_in[:])
                    nc.scalar.dma_start(wo_sb[:], wo_in[:])
                xcache[lb] = (xq, xk, xv)
            return xcache[lb]

        def proj_qk_unit(lb, which, m):
            def fn():
                xq, xk, xv = get_xtiles(lb)
                dst = (qT_sb, kT_sb)[which]
                xs = (xq, xk)[which]
                w_sb = (wq_sb, wk_sb)[which]
                ps = ps_proj.tile([128, LB], F32, tag="ps_proj", name="psp")
                for t in range(ET):
                    lhsT = (w_sb[:, m, t, :] if which == 0
                            else w_sb[:, t, m * 128:(m + 1) * 128])
                    nc.tensor.matmul(
                        ps[:],
                        lhsT=lhsT,
                        rhs=xs[t // 4][:, t % 4, :],
                        start=(t == 0), stop=(t == ET - 1))
                nc.scalar.copy(dst[:, m, lb * LB:(lb + 1) * LB], ps[:])
            return fn

        def proj_v_unit(lb, lt):
            def fn():
                xq, xk, xv = get_xtiles(lb)
                ps = ps_proj.tile([128, HPC, D], F32, tag="ps_proj", name="psv")
                for t in range(ET):
                    nc.tensor.matmul(
                        ps[:],
                        lhsT=xv[t // 4][:, t % 4, lt * 128:(lt + 1) * 128],
                        rhs=wv_sb[:, t, :],
                        start=(t == 0), stop=(t == ET - 1))
                nc.vector.tensor_copy(v_sb[:, lb * 4 + lt, :, 0:D], ps[:])
            return fn

        def oproj_unit(lb, lt):
            def fn():
                l_tile = lb * 4 + lt
                ob = opool.tile([128, 2, LB], BF16, tag="ob", name="ob")
                for e in range(2):
                    ps = ps_proj.tile([128, LB], F32, tag="ps_proj", name="pso")
                    for r in range(MT):
                        nc.tensor.matmul(
                            ps[:],
                            lhsT=oT_sb[:, r, l_tile * 128:(l_tile + 1) * 128],
                            rhs=wo_sb[:, r, e * LB:(e + 1) * LB],
                            start=(r == 0), stop=(r == MT - 1))
                    nc.vector.tensor_copy(ob[:, e, :], ps[:])
                nc.sync.dma_start(out_ext[l_tile], ob[:])
            return fn

        def proj_block(lb):
            for which in range(2):
                for m in range(MT):
                    proj_qk_unit(lb, which, m)()
            for lt in range(LB // 128):
                proj_v_unit(lb, lt)()

        def attn_pair(hp, j, filler):
            nki = 4 * j + 4
            oaccs = [ps_oacc.tile([D + 1, LB], F32, tag="oacc", name="oacc")
                     for _ in range(2)]

            def emit_pv(ki, pT):
                sk = ki - 4 * j
                x0 = 128 * sk if sk >= 0 else 0
                for hi in range(2):
                    nc.tensor.matmul(
                        oaccs[hi][:, x0:LB],
                        lhsT=v_sb[:, ki, 2 * hp + hi, :],
                        rhs=pT[:, hi, x0:LB],
                        start=(ki == 0), stop=(ki == nki - 1))

            prev = None
            for ki in range(nki):
                sk = ki - 4 * j
                x0 = 128 * sk if sk >= 0 else 0
                sp = ps_sp.tile([128, 2, LB], F32, tag="sp", name="sp")
                for hi in range(2):
                    p0 = hi * 64
                    nc.tensor.matmul(
                        sp[:, hi, x0:LB],
                        lhsT=kT_sb[p0:p0 + 64, hp, ki * 128:(ki + 1) * 128],
                        rhs=qT_sb[p0:p0 + 64, hp, j * LB + x0:(j + 1) * LB],
                        start=True, stop=True, tile_position=(p0, 0))
                pT = pTpool.tile([128, 2, LB], BF16, tag="pT", name="pT")
                if not uniform_pad:
                    nc.vector.tensor_scalar_add(
                        sp[:, :, x0:LB], sp[:, :, x0:LB],
                        bias_sb[:, ki:ki + 1])
                nc.scalar.activation(pT[:, :, x0:LB], sp[:, :, x0:LB],
                                     mybir.ActivationFunctionType.Exp,
                                     bias=0.0,
                                     scale=float(EXP_SCALE))
                if sk >= 0:
                    nc.vector.tensor_mul(
                        pT[:, :, x0:x0 + 128],
                        pT[:, :, x0:x0 + 128], trimask[:])
                if prev is not None:
                    emit_pv(*prev)
                prev = (ki, pT)
                filler()
            emit_pv(*prev)
            osbs = []
            for hi in range(2):
                osb = rpool.tile([D + 1, LB], F32, tag="osb", name="osb")
                nc.vector.tensor_copy(osb[:], oaccs[hi][:])
                osbs.append(osb)

            def norm_fn():
                for hi in range(2):
                    osb = osbs[hi]
                    rsum = rpool.tile([1, LB], F32, tag="rsum", name="rsum")
                    nc.vector.tensor_copy(rsum[:], osb[D:D + 1, :])
                    rinv1 = rpool.tile([1, LB], F32, tag="rinv1", name="rinv1")
                    nc.vector.reciprocal_approx_fast(rinv1[:], rsum[:])
                    rinv = rpool.tile([64, LB], F32, tag="rinv", name="rinv")
                    nc.gpsimd.partition_broadcast(rinv[:], rinv1[:])
                    p0 = hi * 64
                    nc.vector.tensor_mul(
                        oT_sb[p0:p0 + 64, hp, j * LB:(j + 1) * LB],
                        osb[0:D, :], rinv[:])
            return norm_fn

        proj_block(0)
        pending_norms = []
        for j in range(NLB):
            units = []
            barriers = {}
            if j < 2:
                for m in range(MT):
                    units.append(proj_qk_unit(j + 1, 0, m))
                    units.append(proj_qk_unit(j + 1, 1, m))
                for lt in range(LB // 128):
                    units.append(proj_v_unit(j + 1, lt))
            elif j == 2:
                for lt in range(LB // 128):
                    units.append(proj_v_unit(3, lt))
                units.append(proj_qk_unit(3, 0, 0))
                units.append(proj_qk_unit(3, 1, 0))
                for lt in range(LB // 128):
                    units.append(oproj_unit(0, lt))
            else:
                for m in range(1, MT):
                    units.append(proj_qk_unit(3, 0, m))
                    units.append(proj_qk_unit(3, 1, m))
                for lt in range(LB // 128):
                    units.append(oproj_unit(1, lt))
                for lt in range(LB // 128):
                    units.append(oproj_unit(2, lt))
                barriers = {1: 2, 2: 4, 3: 6}

            state = {"slot": 0, "done": 0}
            total_slots = (HPC // 2) * (4 * j + 4)

            def filler():
                state["slot"] += 1
                want = len(units) * state["slot"] // total_slots
                while state["done"] < want:
                    units[state["done"]]()
                    state["done"] += 1

            for hp in range(HPC // 2):
                while state["done"] < barriers.get(hp, 0):
                    units[state["done"]]()
                    state["done"] += 1
                nf = attn_pair(hp, j, filler)
                pending_norms.append(nf)
                if len(pending_norms) > 2:
                    pending_norms.pop(0)()
            while state["done"] < len(units):
                units[state["done"]]()
                state["done"] += 1
            while pending_norms:
                pending_norms.pop(0)()
        for lt in range(LB // 128):
            oproj_unit(NLB - 1, lt)()

    nc.compile()
    return nc


_CACHE = {}


def _get_nc(uniform_pad=True):
    key = ("nc", uniform_pad)
    if key not in _CACHE:
        _CACHE[key] = _build(uniform_pad)
    return _CACHE[key]


def _prepare_in_maps(query, key, value, pad_mask, Wq, Wk, Wv, Wo):
    bf = ml_dtypes.bfloat16
    f8 = ml_dtypes.float8_e4m3fn
    query = np.asarray(query, np.float32)
    key = np.asarray(key, np.float32)
    value = np.asarray(value, np.float32)
    pad_mask = np.asarray(pad_mask)
    Wq = np.asarray(Wq, np.float32)
    Wk = np.asarray(Wk, np.float32)
    Wv = np.asarray(Wv, np.float32)
    Wo = np.asarray(Wo, np.float32)

    def tile_act(x):
        # [L, E] -> [E, L] -> [EG, NLB, 128, ET//EG, LB], 4KB/partition chunks
        xt = x.T.reshape(EG, ET // EG, 128, NLB, LB).transpose(0, 3, 2, 1, 4)
        return np.ascontiguousarray(xt.astype(bf))

    per_batch = []
    for b in range(B):
        bias = np.where(pad_mask[b] != 0, 0.0, -30000.0 / EXP_SCALE).astype(np.float32)
        bias = np.ascontiguousarray(bias.reshape(KT, 128).T)
        per_batch.append({
            "qT": tile_act(query[b]),
            "kT": tile_act(key[b]),
            "vT": tile_act(value[b]),
            "bias": bias,
        })

    per_group = []
    for g in range(2):
        sl = slice(g * DH, (g + 1) * DH)
        per_group.append({
            "wq": np.ascontiguousarray(
                Wq[:, sl].astype(bf).reshape(ET, 128, MT, 128).transpose(1, 2, 0, 3)),
            "wk": np.ascontiguousarray(
                Wk[:, sl].astype(bf).reshape(ET, 128, DH).transpose(1, 0, 2)),
            "wv": np.ascontiguousarray(
                Wv[:, sl].astype(bf).reshape(ET, 128, DH).transpose(1, 0, 2)),
            "wo": np.ascontiguousarray(
                Wo[sl, :].astype(bf).reshape(MT, 128, E).transpose(1, 0, 2)),
        })

    in_maps = []
    for b in range(B):
        for g in range(2):
            m = dict(per_batch[b])
            m.update(per_group[g])
            in_maps.append(m)
    return in_maps


def _combine(results):
    out = np.empty((B, L, E), np.float32)
    for b in range(B):
        acc = (results[2 * b]["out"].astype(np.float32)
               + results[2 * b + 1]["out"].astype(np.float32))
        out[b] = acc.reshape(L, E)
    return out


def kernel(query, key, value, pad_mask, Wq, Wk, Wv, Wo):
    nc = _get_nc(bool(np.all(np.asarray(pad_mask) != 0)))
    in_maps = _prepare_in_maps(query, key, value, pad_mask, Wq, Wk, Wv, Wo)
    res = run_bass_kernel_spmd(nc, in_maps, core_ids=list(range(NCORES)))
    return _combine(res.results)
